# revision 4
# baseline (speedup 1.0000x reference)
"""Trainium2 Bass kernel for windowed (banded) self-attention MLP block.

Reference computation (per batch b):
    h = relu(x @ W1 + b1)                      # [S, H]
    q = h @ Wq                                 # [S, H]
    scores[s, w] = q[s] . h_pad[s + w] / 32    # window w in [0, 33), h zero-padded by A=16
    wgt = softmax(scores, axis=w)
    out[s] = sum_w wgt[s, w] * h_pad[s + w]

Sharding: 8 cores, each takes 1024 consecutive tokens of the flattened
[B*S] = 8192 token stream (2 cores per batch element; shards never cross a
batch boundary).  Each core redundantly computes h for a 16-token halo on
each side, so no cross-core communication is needed.

Per-core layouts (host prepares everything transposed/chunked; 'aug' = the
bias trick: x gains a validity row of 1.0s and W1 gains the b1 row, so
h = relu(x_aug @ W1_aug) and out-of-range halo tokens come out exactly 0):
    xa  [128, 5, 1152] bf16   x_aug^T chunked along IN (4 chunks + aug chunk)
    w1  [128, 5, 1024] bf16   W1_aug chunked along IN
    wq  [128, 8, 1024] bf16   (Wq / 32) chunked along H_in
    out [1024, 1024]   f32

On-chip stages (all matmuls bf16 operands, fp32 PSUM accumulation):
    A:  hT[hc, t]  = relu(W1_aug^T @ xT_aug)     H-on-partitions, 1056 tokens
    B:  qT[ho, t]  = (Wq/32)^T @ hT              core 1024 tokens
    C:  h[t, hc]   = relu(xT_aug^T @ W1_aug)     token-on-partitions (recompute
                                                 instead of transposing hT)
    D:  per 128-token tile: scores = qT^T @ hT_window  [128, 160]
        p = exp(scores + bandmask), denominator via ACT accum_out
        pT via PE transpose; out = (pT^T @ h_window) * (1/den)
"""

import os
import sys

import numpy as np

if "/opt/trn_rl_repo" not in sys.path:
    sys.path.insert(0, "/opt/trn_rl_repo")

import ml_dtypes

import concourse.bass as bass
import concourse.mybir as mybir
import concourse.tile as tile
from concourse import bacc
from concourse.bass_utils import run_bass_kernel_spmd

BF16 = ml_dtypes.bfloat16

B, S, IN, H = 4, 2048, 512, 1024
A = 16
WND = 2 * A + 1            # 33 window positions
NCORES = 8
TOK = (B * S) // NCORES    # 1024 tokens per core
TOKH = TOK + 2 * A         # 1056 with halo
TOKP = 9 * 128             # 1152 zero-padded token slots
NT = TOK // 128            # 8 output tiles per core
WIN = 128 + 2 * A          # 160-token window per 128-token tile
NEG = -30000.0             # additive mask for out-of-band positions

f32 = mybir.dt.float32
bf16 = mybir.dt.bfloat16
AF = mybir.ActivationFunctionType


def _band_mask():
    """[128, WIN] additive mask: row t allows window cols t..t+32."""
    m = np.full((128, WIN), NEG, dtype=np.float32)
    for t in range(128):
        m[t, t : t + WND] = 0.0
    return m


def _kernel_body(tc, nc, xa_d, w1_d, wq_d, out_d, mask_d, id_d):
    with (
        tc.tile_pool(name="const", bufs=1) as cpool,
        tc.tile_pool(name="wts", bufs=1) as wpool,
        tc.tile_pool(name="acts", bufs=1) as apool,
    ):
        mask_sb = cpool.tile([128, WIN], f32, tag="mask")
        nc.sync.dma_start(mask_sb[:], mask_d[:])
        id_sb = cpool.tile([128, 128], bf16, tag="ident")
        nc.sync.dma_start(id_sb[:], id_d[:])

        xa = wpool.tile([128, 5, TOKP], bf16, tag="xa")
        nc.sync.dma_start(xa[:], xa_d[:])
        w1 = wpool.tile([128, 5, H], bf16, tag="w1")
        nc.sync.dma_start(w1[:], w1_d[:])
        wq = wpool.tile([128, 8, H], bf16, tag="wq")
        nc.sync.dma_start(wq[:], wq_d[:])

        hT = apool.tile([128, 8, TOKH], bf16, tag="hT")
        hh = apool.tile([128, 9, H], bf16, tag="hh")
        qT = apool.tile([128, 8, TOK], bf16, tag="qT")

        # ---- stages A, B, C (big dense matmuls) ----
        with tc.tile_pool(name="psABC", bufs=1, space="PSUM") as psABC:
            # A: hT = relu(W1_aug^T @ xT_aug), 3 token tiles of 352
            ATILE = 352
            for t in range(3):
                sl = slice(t * ATILE, (t + 1) * ATILE)
                for hc in range(8):
                    ps = psABC.tile([128, ATILE], f32, tag="pa", bufs=2)
                    for c in range(5):
                        nc.tensor.matmul(
                            ps[:],
                            w1[:, c, hc * 128 : (hc + 1) * 128],
                            xa[:, c, sl],
                            start=(c == 0),
                            stop=(c == 4),
                        )
                    nc.scalar.activation(hT[:, hc, sl], ps[:], AF.Relu)

            # C: h = relu(xT_aug^T @ W1_aug), 9 token tiles of 128
            for t in range(9):
                tsl = slice(t * 128, (t + 1) * 128)
                for half in range(2):
                    ps = psABC.tile([128, 512], f32, tag="pc", bufs=2)
                    for c in range(5):
                        nc.tensor.matmul(
                            ps[:],
                            xa[:, c, tsl],
                            w1[:, c, half * 512 : (half + 1) * 512],
                            start=(c == 0),
                            stop=(c == 4),
                        )
                    nc.scalar.activation(
                        hh[:, t, half * 512 : (half + 1) * 512], ps[:], AF.Relu
                    )

            # B: qT = (Wq/32)^T @ hT for the core 1024 tokens
            for th in range(2):
                off = th * 512
                for ho in range(8):
                    ps = psABC.tile([128, 512], f32, tag="pb", bufs=2)
                    for hi in range(8):
                        nc.tensor.matmul(
                            ps[:],
                            wq[:, hi, ho * 128 : (ho + 1) * 128],
                            hT[:, hi, A + off : A + off + 512],
                            start=(hi == 0),
                            stop=(hi == 7),
                        )
                    nc.scalar.activation(qT[:, ho, off : off + 512], ps[:], AF.Copy)

        # ---- stage D: windowed attention per 128-token tile ----
        with (
            tc.tile_pool(name="psD", bufs=1, space="PSUM") as psD,
            tc.tile_pool(name="dtmp", bufs=2) as dpool,
            tc.tile_pool(name="outp", bufs=3) as opool,
        ):
            for T in range(NT):
                ps_s = psD.tile([128, WIN], f32, tag="ps", bufs=2)
                for hc in range(8):
                    nc.tensor.matmul(
                        ps_s[:],
                        qT[:, hc, T * 128 : (T + 1) * 128],
                        hT[:, hc, T * 128 : T * 128 + WIN],
                        start=(hc == 0),
                        stop=(hc == 7),
                    )
                s_sb = dpool.tile([128, WIN], f32, tag="s")
                nc.vector.tensor_add(s_sb[:], ps_s[:], mask_sb[:])
                p_sb = dpool.tile([128, WIN], bf16, tag="p")
                den = dpool.tile([128, 1], f32, tag="den")
                nc.scalar.activation(p_sb[:], s_sb[:], AF.Exp, accum_out=den[:])
                rcp = dpool.tile([128, 1], f32, tag="rcp")
                nc.vector.reciprocal(rcp[:], den[:])

                ptA = psD.tile([128, 128], bf16, tag="pt", bufs=2)
                nc.tensor.transpose(ptA[:], p_sb[:, 0:128], id_sb[:])
                ptB = psD.tile([32, 128], bf16, tag="ptb", bufs=2)
                nc.tensor.transpose(ptB[:], p_sb[:, 128:WIN], id_sb[:])
                pta_sb = dpool.tile([128, 128], bf16, tag="pta")
                nc.vector.tensor_copy(pta_sb[:], ptA[:])
                ptb_sb = dpool.tile([32, 128], bf16, tag="ptb_sb")
                nc.vector.tensor_copy(ptb_sb[:], ptB[:])

                out_sb = opool.tile([128, H], f32, tag="osb")
                for half in range(2):
                    hsl = slice(half * 512, (half + 1) * 512)
                    pav = psD.tile([128, 512], f32, tag="pav", bufs=2)
                    nc.tensor.matmul(
                        pav[:], pta_sb[:], hh[:, T, hsl], start=True, stop=False
                    )
                    nc.tensor.matmul(
                        pav[:], ptb_sb[:], hh[0:32, T + 1, hsl], start=False, stop=True
                    )
                    nc.vector.tensor_scalar_mul(out_sb[:, hsl], pav[:], rcp[:])
                nc.sync.dma_start(out_d[T * 128 : (T + 1) * 128, :], out_sb[:])


def build_nc():
    nc = bacc.Bacc("TRN2", target_bir_lowering=False, debug=False, num_devices=NCORES)
    xa_d = nc.dram_tensor("xa", [128, 5, TOKP], bf16, kind="ExternalInput")
    w1_d = nc.dram_tensor("w1", [128, 5, H], bf16, kind="ExternalInput")
    wq_d = nc.dram_tensor("wq", [128, 8, H], bf16, kind="ExternalInput")
    out_d = nc.dram_tensor("out", [TOK, H], f32, kind="ExternalOutput")
    mask_d = nc.inline_tensor(_band_mask(), "bandmask")
    id_d = nc.inline_tensor(np.eye(128, dtype=BF16), "ident")

    with tile.TileContext(nc) as tc:
        _kernel_body(tc, nc, xa_d, w1_d, wq_d, out_d, mask_d, id_d)
    nc.compile()
    return nc


def make_inputs(x, W1, b1, Wq):
    """Host-side shard prep (numpy only; not part of HW time)."""
    x = np.asarray(x, dtype=np.float32)
    W1 = np.asarray(W1, dtype=np.float32)
    b1 = np.asarray(b1, dtype=np.float32)
    Wq = np.asarray(Wq, dtype=np.float32)

    w1a = np.zeros((128, 5, H), dtype=BF16)
    for c in range(4):
        w1a[:, c, :] = W1[c * 128 : (c + 1) * 128, :].astype(BF16)
    w1a[0, 4, :] = b1.astype(BF16)

    wqs = (Wq / np.sqrt(np.float32(H))).astype(BF16)
    wqa = np.zeros((128, 8, H), dtype=BF16)
    for c in range(8):
        wqa[:, c, :] = wqs[c * 128 : (c + 1) * 128, :]

    in_maps = []
    for core in range(NCORES):
        b, half = divmod(core, 2)
        lo = half * TOK - A
        hi = half * TOK + TOK + A
        s0, s1 = max(lo, 0), min(hi, S)
        xs = np.zeros((TOKH, IN), dtype=np.float32)
        xs[s0 - lo : s1 - lo] = x[b, s0:s1]
        xT = np.ascontiguousarray(xs.T).astype(BF16)  # [512, 1056]
        xa = np.zeros((128, 5, TOKP), dtype=BF16)
        for c in range(4):
            xa[:, c, :TOKH] = xT[c * 128 : (c + 1) * 128, :]
        xa[0, 4, s0 - lo : s1 - lo] = BF16(1.0)
        in_maps.append({"xa": xa, "w1": w1a, "wq": wqa})
    return in_maps


_NC_CACHE = {}


def get_nc():
    if "nc" not in _NC_CACHE:
        _NC_CACHE["nc"] = build_nc()
    return _NC_CACHE["nc"]


def kernel(x, W1, b1, Wq, atten_size, _trace=False, _trace_kwargs=None):
    assert int(atten_size) == A, f"kernel hardcodes atten_size=16, got {atten_size}"
    nc = get_nc()
    in_maps = make_inputs(x, W1, b1, Wq)
    kw = {}
    if _trace:
        kw = dict(trace=True, trace_kwargs=_trace_kwargs or {})
    res = run_bass_kernel_spmd(nc, in_maps, core_ids=list(range(NCORES)), **kw)
    out = np.stack([r["out"] for r in res.results])  # [8, 1024, 1024]
    out = out.reshape(B, S, H)
    if _trace:
        return out, res
    return out


if __name__ == "__main__":
    import jax

    key = jax.random.key(0)
    k1, k2, k3, k4 = jax.random.split(key, 4)
    x = np.asarray(jax.random.normal(k1, (B, S, IN), dtype=np.float32))
    W1 = np.asarray(
        jax.random.normal(k2, (IN, H), dtype=np.float32) * (1.0 / np.sqrt(IN))
    )
    b1 = np.asarray(jax.random.normal(k3, (H,), dtype=np.float32) * 0.02)
    Wq = np.asarray(
        jax.random.normal(k4, (H, H), dtype=np.float32) * (1.0 / np.sqrt(H))
    )
    out = kernel(x, W1, b1, Wq, 16)
    print("out", out.shape, out.dtype, float(np.abs(out).max()))
